# revision 9
# baseline (speedup 1.0000x reference)
"""Trainium2 Bass kernel for nn_ActQuantWrapper (per-token 4-bit fake-quant + Linear).

Strategy (8 NeuronCores, SPMD, no collectives):
  - Shard x along the sequence axis: 1024 tokens per core; weight/bias replicated.
  - Host prep: features PERMUTED so the 3840 quantized features are columns
    [0:3840) and the 256 fp features are [3840:4096). The contraction
    mixed @ W^T is invariant under a common permutation of x-columns and
    W-columns, so no masks / copy_predicated / scatter are needed on device.
    W^T is pre-permuted and cast to bf16 on host.
  - Per core, per 128-token tile:
      * DVE: min/max reduces over the q-columns (then clamped with 0 to match
        the reference's min(.,0)/max(.,0)); per-token params in [128,1] tiles
      * ACT: round pass in place on the q-columns via the RNE +/-MAGIC trick
        fused into activation(x*inv + MAGIC)
      * DVE: clip (dual-op sub/min), then (max,mult) producing bf16 dq directly
        into mixed16[:, :3840]; fp columns copied as bf16(x) into [3840:4096)
      * DMA-xbar transpose (ACT queue) into mixed^T [feature, token] tiles
  - Matmul: stationary = mixed^T tile (128x128), moving = W^T chunk (N=512),
    PSUM accum over 32 feature tiles; DVE adds broadcast bias on drain.
    Two token groups (384 + 640 tokens): chosen so the first group's W
    streaming rate (32 MiB per group sweep) plus x loads stays under the
    per-core HBM bandwidth; W^T streams once per group (64 MiB total).
  - DMA issue streams: x0-x2 then W chunks on Sync (HWDGE), later x tiles +
    transposes on Scalar/ACT (HWDGE, issued right after the prior transpose
    so pool WAR waits don't block the queue), bias + outputs on GpSimd.
"""

import sys
import numpy as np
import ml_dtypes

sys.path.insert(0, "/opt/trn_rl_repo")

import concourse.bass as bass  # noqa: E402
import concourse.mybir as mybir  # noqa: E402
import concourse.tile as tile  # noqa: E402
from concourse import bacc  # noqa: E402

F32 = mybir.dt.float32
BF16 = mybir.dt.bfloat16

N_CORES = 8
S_FULL, D, O = 8192, 4096, 4096
DQ = 3840                      # quantized features, permuted to the front
T = S_FULL // N_CORES          # tokens per core
MAGIC = 12582912.0             # 1.5 * 2**23 : RNE round-to-int for |v| < 2**22
MAXQ = 15.0
RANGE_FLOOR = 1e-30            # degenerate all-zero token guard (dq ends up 0 anyway)

N_TT = T // 128                # token tiles per core
GROUP_TTS = [3, 5]             # token tiles per group (sums to N_TT)
CHUNK = 512                    # output-feature chunk per W^T stream tile
N_CH = O // CHUNK
N_DT = D // 128                # feature (contraction) tiles
MT_BUFS = 8                    # live mixed^T tiles
WC_BUFS = 2                    # W chunk prefetch depth
XP_BUFS = 3                    # x tile prefetch depth

_CACHE = {}


def _build_bass():
    nc = bacc.Bacc("TRN2", target_bir_lowering=False, debug=False,
                   enable_asserts=True, num_devices=N_CORES)
    x_ap = nc.dram_tensor("x", [T, D], F32, kind="ExternalInput").ap()
    wt_ap = nc.dram_tensor("wt", [D, O], BF16, kind="ExternalInput").ap()
    bf_ap = nc.dram_tensor("biasf", [1, O], F32, kind="ExternalInput").ap()
    out_ap = nc.dram_tensor("out", [T, O], F32, kind="ExternalOutput").ap()

    with tile.TileContext(nc) as tc:
        _kernel_body(tc, out_ap, x_ap, wt_ap, bf_ap)
    nc.compile()
    return nc


def _kernel_body(tc, out_ap, x_ap, wt_ap, bf_ap):
    from contextlib import ExitStack
    nc = tc.nc
    A = mybir.AluOpType
    AF = mybir.ActivationFunctionType

    with ExitStack() as ctx:
        xp = ctx.enter_context(tc.tile_pool(name="xp", bufs=XP_BUFS))
        mxp = ctx.enter_context(tc.tile_pool(name="mxp", bufs=2))
        pp = ctx.enter_context(tc.tile_pool(name="pp", bufs=2))
        mtp = ctx.enter_context(tc.tile_pool(name="mtp", bufs=MT_BUFS))
        wcp = ctx.enter_context(tc.tile_pool(name="wcp", bufs=WC_BUFS))
        bbp = ctx.enter_context(tc.tile_pool(name="bbp", bufs=2))
        osp = ctx.enter_context(tc.tile_pool(name="osp", bufs=2))
        pmm = ctx.enter_context(tc.tile_pool(name="pmm", bufs=4, space="PSUM"))

        def load_wtc(ch):
            col = ch * CHUNK
            wtc = wcp.tile([128, N_DT, CHUNK], BF16, tag="wtc")
            nc.sync.dma_start(
                out=wtc,
                in_=wt_ap[0:D, col:col + CHUNK].rearrange("(j p) c -> p j c", p=128))
            bias_b = bbp.tile([128, CHUNK], F32, tag="bb")
            nc.gpsimd.dma_start(out=bias_b, in_=bass.AP(
                tensor=bf_ap.tensor, offset=bf_ap.offset + col,
                ap=[[0, 128], [1, CHUNK]]))
            return wtc, bias_b

        # x tiles on HWDGE queues: the first XP_BUFS go on sync ahead of the
        # first W chunk (quant pipeline never starved early); later tiles are
        # issued from the scalar queue right after the previous transpose so
        # their pool-slot WAR wait is short and W streaming is not blocked.
        xts = {}

        def load_x(tti, eng):
            xt = xp.tile([128, D], F32, tag="x")
            eng.dma_start(out=xt, in_=x_ap[tti * 128:(tti + 1) * 128, :])
            xts[tti] = xt

        for tti in range(XP_BUFS):
            load_x(tti, nc.sync)

        # chunk schedule: per group, all 8 W chunks; 16 loads total, 2 prefetched
        chunk_seq = [ch for _ in GROUP_TTS for ch in range(N_CH)]
        wtcs = {}
        for k in range(WC_BUFS):
            wtcs[k] = load_wtc(chunk_seq[k])

        mts = {}                      # global token-tile index -> mixed^T tile
        row0 = 0
        kseq = 0                      # position in chunk_seq
        for g, g_tts in enumerate(GROUP_TTS):
            for tt in range(g_tts):
                tti = row0 // 128 + tt
                row = row0 + tt * 128
                xt = xts.pop(tti)
                xq = xt[:, 0:DQ]

                # per-token stats over q-columns, clamped with 0 (reference
                # uses min(qf.min, 0) / max(qf.max, 0))
                rmax = pp.tile([128, 1], F32, tag="rmax")
                rmin = pp.tile([128, 1], F32, tag="rmin")
                nc.vector.tensor_reduce(rmax, xq, axis=mybir.AxisListType.X, op=A.max)
                nc.vector.tensor_reduce(rmin, xq, axis=mybir.AxisListType.X, op=A.min)
                rmax0 = pp.tile([128, 1], F32, tag="rmax0")
                rmin0 = pp.tile([128, 1], F32, tag="rmin0")
                nc.vector.tensor_scalar(rmax0, rmax, 0.0, None, A.max)
                nc.vector.tensor_scalar(rmin0, rmin, 0.0, None, A.min)

                rng = pp.tile([128, 1], F32, tag="rng")
                nc.vector.tensor_tensor(rng, rmax0, rmin0, A.subtract)
                s = pp.tile([128, 1], F32, tag="s")       # scale = range/15
                nc.vector.tensor_scalar(s, rng, RANGE_FLOOR, 1.0 / MAXQ, A.max, A.mult)
                inv = pp.tile([128, 1], F32, tag="inv")
                nc.vector.reciprocal(inv, s)
                lop = pp.tile([128, 1], F32, tag="lop")   # lo = round(xmin/scale) = -zero
                nc.vector.tensor_scalar(lop, rmin0, inv, MAGIC, A.mult, A.add)
                lo = pp.tile([128, 1], F32, tag="lo")
                nc.vector.tensor_scalar(lo, lop, MAGIC, None, A.subtract)
                hi = pp.tile([128, 1], F32, tag="hi")
                nc.vector.tensor_scalar(hi, lo, MAXQ, None, A.add)

                # quantize in place on the q-columns:
                #   ACT: xq <- xq*inv + MAGIC   (RNE round-to-int in the mantissa)
                #   DVE: xq <- min(xq - MAGIC, hi) ; mixed16 <- bf16(max(xq, lo) * s)
                nc.scalar.activation(xq, xq, AF.Copy, bias=MAGIC, scale=inv)
                nc.vector.tensor_scalar(xq, xq, MAGIC, hi, A.subtract, A.min)
                mixed16 = mxp.tile([128, D], BF16, tag="mx")
                nc.vector.tensor_scalar(mixed16[:, 0:DQ], xq, lo, s, A.max, A.mult)
                # fp columns keep x (cast to bf16)
                nc.vector.tensor_copy(mixed16[:, DQ:D], xt[:, DQ:D])

                # DMA-xbar block-transpose: mt[p, j, t] = mixed16[t, 128*j + p]
                mt = mtp.tile([128, N_DT, 128], BF16, tag="mt")
                mts[tti] = mt
                nc.scalar.dma_start_transpose(mt, mixed16)
                if tti + XP_BUFS < N_TT:
                    load_x(tti + XP_BUFS, nc.scalar)

            # matmul phase: stream W^T chunks, accumulate over feature tiles
            for ch in range(N_CH):
                col = ch * CHUNK
                wtc, bias_b = wtcs.pop(kseq)
                if kseq + WC_BUFS < len(chunk_seq):
                    wtcs[kseq + WC_BUFS] = load_wtc(chunk_seq[kseq + WC_BUFS])
                kseq += 1

                for tt in range(g_tts):
                    tti = row0 // 128 + tt
                    row = row0 + tt * 128
                    ps = pmm.tile([128, CHUNK], F32, tag="mm")
                    for j in range(N_DT):
                        nc.tensor.matmul(ps, lhsT=mts[tti][:, j, :], rhs=wtc[:, j, :],
                                         start=(j == 0), stop=(j == N_DT - 1))
                    ost = osp.tile([128, CHUNK], F32, tag="ost")
                    nc.vector.tensor_tensor(ost, ps, bias_b, A.add)
                    nc.gpsimd.dma_start(out=out_ap[row:row + 128, col:col + CHUNK],
                                        in_=ost)
            row0 += g_tts * 128


def _get_nc():
    if "nc" not in _CACHE:
        _CACHE["nc"] = _build_bass()
    return _CACHE["nc"]


def _prep_in_maps(x, weight, bias, q_idx, fp_idx):
    x = np.ascontiguousarray(np.asarray(x, dtype=np.float32)).reshape(S_FULL, D)
    weight = np.asarray(weight, dtype=np.float32)
    bias = np.asarray(bias, dtype=np.float32)
    q_idx = np.asarray(q_idx).astype(np.int64)
    fp_idx = np.asarray(fp_idx).astype(np.int64)

    perm = np.concatenate([q_idx, fp_idx])
    xp = np.ascontiguousarray(x[:, perm])
    wt = np.ascontiguousarray(weight[:, perm].T.astype(ml_dtypes.bfloat16))

    shared = {"wt": wt, "biasf": np.ascontiguousarray(bias[None, :])}
    return [
        {"x": np.ascontiguousarray(xp[c * T:(c + 1) * T]), **shared}
        for c in range(N_CORES)
    ]


def kernel(x, weight, bias, q_idx, fp_idx):
    from concourse import bass_utils
    bass_utils.upload_artifacts = lambda tmpdir: "local://none"

    nc = _get_nc()
    in_maps = _prep_in_maps(x, weight, bias, q_idx, fp_idx)
    res = bass_utils.run_bass_kernel_spmd(
        nc, in_maps, core_ids=list(range(N_CORES)))
    out = np.concatenate([res.results[c]["out"] for c in range(N_CORES)], axis=0)
    return out.reshape(1, S_FULL, O)


# revision 17
# speedup vs baseline: 1.1624x; 1.1624x over previous
"""Trainium2 Bass kernel for nn_ActQuantWrapper (per-token 4-bit fake-quant + Linear).

Strategy (8 NeuronCores, SPMD, no collectives):
  - Shard x along the sequence axis: 1024 tokens per core; weight/bias replicated.
  - Host prep: features PERMUTED so the 3840 quantized features are columns
    [0:3840) and the 256 fp features are [3840:4096). The contraction
    mixed @ W^T is invariant under a common permutation of x-columns and
    W-columns, so no masks / copy_predicated / scatter are needed on device.
    W^T is pre-permuted and cast to bf16 on host.
  - Per core, per 128-token tile:
      * x loaded as two half-DMAs; DVE min/max reduces run per half and are
        combined (with the reference's min(.,0)/max(.,0) clamp) on GpSimd
      * the whole per-token param chain ([128,1] ops incl. 1/s via an exact
        IEEE divide) runs on GpSimd so it never queues behind the next
        tile's 4-us reduces on the Vector engine
      * ACT: round pass in place on the q-columns via the RNE +/-MAGIC trick
        fused into activation(x*inv + MAGIC)
      * DVE: clip (dual-op sub/min), then (max,mult) producing bf16 dq into
        mixed16[:, :3840]; fp columns copied as bf16(x) into [3840:4096)
      * DMA-xbar transpose (ACT queue) into mixed^T [feature, token] tiles
  - Matmul: stationary = mixed^T tile (128x128), moving = W^T chunk (N=512),
    PSUM accum over 32 feature tiles; DVE adds broadcast bias on drain.
    The (token-tile x out-chunk) grid is covered by RECTANGLE PHASES so the
    W^T streaming rate stays smooth and low while matmuls start after only
    ~2 tile-quants:  [t0-t2 x c0-c1], [t0-t4 x c2-c3], [all x c4-c7],
    [t3-t7 x c0-c1], [t5-t7 x c2-c3]  (48 MiB of W traffic total).
    Each W chunk is DMA'd as 4 sub-tiles so matmuls begin when the first
    quarter lands and the prefetch ring refills during the burst itself.
  - DMA issue streams: x0-x2 halves then W sub-chunks on Sync (HWDGE),
    later x tiles + transposes on Scalar/ACT (HWDGE), bias + param-chain +
    outputs on GpSimd.
"""

import sys
import numpy as np
import ml_dtypes

sys.path.insert(0, "/opt/trn_rl_repo")

import concourse.bass as bass  # noqa: E402
import concourse.mybir as mybir  # noqa: E402
import concourse.tile as tile  # noqa: E402
from concourse import bacc  # noqa: E402

F32 = mybir.dt.float32
BF16 = mybir.dt.bfloat16

N_CORES = 8
S_FULL, D, O = 8192, 4096, 4096
DQ = 3840                      # quantized features, permuted to the front
XH = 2048                      # x half-load split column
T = S_FULL // N_CORES          # tokens per core
MAGIC = 12582912.0             # 1.5 * 2**23 : RNE round-to-int for |v| < 2**22
MAXQ = 15.0
RANGE_FLOOR = 1e-30            # degenerate all-zero token guard (dq ends up 0 anyway)

N_TT = T // 128                # token tiles per core
CHUNK = 512                    # output-feature chunk per W^T stream tile
N_CH = O // CHUNK
N_DT = D // 128                # feature (contraction) tiles
NSUB = 4                       # W sub-tiles per chunk
JSUB = N_DT // NSUB            # feature tiles per W sub-tile
XP_BUFS = 3
WC_ENT = 2                     # W chunk entries in flight (x NSUB sub-tiles)

# Burst schedule covering the 8x8 (token-tile x chunk) grid: entries are W
# chunk loads in order; SCHED lists (entry index, token tiles) PE bursts.
# Entries 0/1 are prefetched and each serves two early bursts so matmuls
# start after a single tile-quant; later entries stream in via the ring.
ENTRY_CH = [0, 1, 2, 3, 4, 5, 6, 7, 0, 1, 2, 3]   # chunk per W-load entry
SCHED = [
    (0, [0]), (1, [0, 1]), (0, [1, 2]), (1, [2]),
    (2, [0, 1, 2, 3, 4]), (3, [0, 1, 2, 3, 4]),
    (4, list(range(8))), (5, list(range(8))),
    (6, list(range(8))), (7, list(range(8))),
    (8, [3, 4, 5, 6, 7]), (9, [3, 4, 5, 6, 7]),
    (10, [5, 6, 7]), (11, [5, 6, 7]),
]
# position after which each entry is dead -> trigger for loading entry k+2
_LAST_USE = {e: p for p, (e, _) in enumerate(SCHED)}

_CACHE = {}


def _build_bass():
    nc = bacc.Bacc("TRN2", target_bir_lowering=False, debug=False,
                   enable_asserts=True, num_devices=N_CORES)
    x_ap = nc.dram_tensor("x", [T, D], F32, kind="ExternalInput").ap()
    wt_ap = nc.dram_tensor("wt", [D, O], BF16, kind="ExternalInput").ap()
    bf_ap = nc.dram_tensor("biasf", [1, O], F32, kind="ExternalInput").ap()
    out_ap = nc.dram_tensor("out", [T, O], F32, kind="ExternalOutput").ap()

    with tile.TileContext(nc) as tc:
        _kernel_body(tc, out_ap, x_ap, wt_ap, bf_ap)
    nc.compile()
    return nc


def _kernel_body(tc, out_ap, x_ap, wt_ap, bf_ap):
    from contextlib import ExitStack
    nc = tc.nc
    A = mybir.AluOpType
    AF = mybir.ActivationFunctionType

    with ExitStack() as ctx:
        xp = ctx.enter_context(tc.tile_pool(name="xp", bufs=XP_BUFS))
        mxp = ctx.enter_context(tc.tile_pool(name="mxp", bufs=2))
        pp = ctx.enter_context(tc.tile_pool(name="pp", bufs=2))
        mtp = ctx.enter_context(tc.tile_pool(name="mtp", bufs=N_TT))
        wcp = ctx.enter_context(tc.tile_pool(name="wcp", bufs=WC_ENT * NSUB))
        bbp = ctx.enter_context(tc.tile_pool(name="bbp", bufs=3))
        osp = ctx.enter_context(tc.tile_pool(name="osp", bufs=2))
        pmm = ctx.enter_context(tc.tile_pool(name="pmm", bufs=4, space="PSUM"))

        def load_wtc(ch):
            col = ch * CHUNK
            subs = []
            for sub in range(NSUB):
                wts = wcp.tile([128, JSUB, CHUNK], BF16, tag="wts")
                r0 = sub * JSUB * 128
                nc.sync.dma_start(
                    out=wts,
                    in_=wt_ap[r0:r0 + JSUB * 128, col:col + CHUNK]
                    .rearrange("(j p) c -> p j c", p=128))
                subs.append(wts)
            bias_b = bbp.tile([128, CHUNK], F32, tag="bb")
            nc.gpsimd.dma_start(out=bias_b, in_=bass.AP(
                tensor=bf_ap.tensor, offset=bf_ap.offset + col,
                ap=[[0, 128], [1, CHUNK]]))
            return subs, bias_b

        # x tiles: two half-DMAs each; first XP_BUFS on sync ahead of W so the
        # quant pipeline starts immediately; later tiles from the scalar queue
        # right after the prior transpose (pool WAR wait is then short).
        xts = {}

        def load_x(tti, eng):
            xt = xp.tile([128, D], F32, tag="x")
            row = tti * 128
            eng.dma_start(out=xt[:, 0:XH], in_=x_ap[row:row + 128, 0:XH])
            eng.dma_start(out=xt[:, XH:D], in_=x_ap[row:row + 128, XH:D])
            xts[tti] = xt

        for tti in range(XP_BUFS):
            load_x(tti, nc.sync)

        # W prefetch ring over the entry sequence
        wtcs = {}
        for k in range(WC_ENT):
            wtcs[k] = load_wtc(ENTRY_CH[k])

        mts = {}
        done_quant = 0

        def quant_tile(tti):
            row = tti * 128
            xt = xts.pop(tti)
            # per-token stats: half reduces on DVE, combine + clamp-0 on GpSimd
            rmaxa = pp.tile([128, 1], F32, tag="rmaxa")
            rmina = pp.tile([128, 1], F32, tag="rmina")
            rmaxb = pp.tile([128, 1], F32, tag="rmaxb")
            rminb = pp.tile([128, 1], F32, tag="rminb")
            nc.vector.tensor_reduce(rmaxa, xt[:, 0:XH], axis=mybir.AxisListType.X, op=A.max)
            nc.vector.tensor_reduce(rmina, xt[:, 0:XH], axis=mybir.AxisListType.X, op=A.min)
            nc.vector.tensor_reduce(rmaxb, xt[:, XH:DQ], axis=mybir.AxisListType.X, op=A.max)
            nc.vector.tensor_reduce(rminb, xt[:, XH:DQ], axis=mybir.AxisListType.X, op=A.min)
            # param chain ([128,1] DVE ops, AP scalars fold the half-combines
            # and the 0-clamp into single dual-op instructions)
            rmax = pp.tile([128, 1], F32, tag="rmax")
            rmin = pp.tile([128, 1], F32, tag="rmin")
            nc.vector.tensor_scalar(rmax, rmaxa, rmaxb, 0.0, A.max, A.max)
            nc.vector.tensor_scalar(rmin, rmina, rminb, 0.0, A.min, A.min)
            rng = pp.tile([128, 1], F32, tag="rng")
            nc.vector.tensor_tensor(rng, rmax, rmin, A.subtract)
            s = pp.tile([128, 1], F32, tag="s")       # scale = range/15
            nc.vector.tensor_scalar(s, rng, RANGE_FLOOR, 1.0 / MAXQ, A.max, A.mult)
            inv = pp.tile([128, 1], F32, tag="inv")
            nc.vector.reciprocal(inv, s)
            lop = pp.tile([128, 1], F32, tag="lop")   # lo = round(xmin/scale) = -zero
            nc.vector.tensor_scalar(lop, rmin, inv, MAGIC, A.mult, A.add)
            lo = pp.tile([128, 1], F32, tag="lo")
            nc.vector.tensor_scalar(lo, lop, MAGIC, None, A.subtract)
            hi = pp.tile([128, 1], F32, tag="hi")
            nc.vector.tensor_scalar(hi, lo, MAXQ, None, A.add)

            # quantize in place on the q-columns:
            #   ACT: xq <- xq*inv + MAGIC   (RNE round-to-int in the mantissa)
            #   DVE: xq <- min(xq - MAGIC, hi) ; mixed16 <- bf16(max(xq, lo) * s)
            xq = xt[:, 0:DQ]
            nc.scalar.activation(xq, xq, AF.Copy, bias=MAGIC, scale=inv)
            nc.vector.tensor_scalar(xq, xq, MAGIC, hi, A.subtract, A.min)
            mixed16 = mxp.tile([128, D], BF16, tag="mx")
            nc.vector.tensor_scalar(mixed16[:, 0:DQ], xq, lo, s, A.max, A.mult)
            nc.vector.tensor_copy(mixed16[:, DQ:D], xt[:, DQ:D])

            # DMA-xbar block-transpose: mt[p, j, t] = mixed16[t, 128*j + p]
            mt = mtp.tile([128, N_DT, 128], BF16, tag="mt")
            mts[tti] = mt
            nc.scalar.dma_start_transpose(mt, mixed16)
            nxt = tti + XP_BUFS
            if nxt < N_TT:
                load_x(nxt, nc.scalar)

        for pos, (ent, tts) in enumerate(SCHED):
            # quant any tiles this burst needs that aren't done yet
            while done_quant <= max(tts):
                quant_tile(done_quant)
                done_quant += 1

            col = ENTRY_CH[ent] * CHUNK
            subs, bias_b = wtcs[ent]

            for tti in tts:
                row = tti * 128
                ps = pmm.tile([128, CHUNK], F32, tag="mm")
                for j in range(N_DT):
                    nc.tensor.matmul(ps, lhsT=mts[tti][:, j, :],
                                     rhs=subs[j // JSUB][:, j % JSUB, :],
                                     start=(j == 0), stop=(j == N_DT - 1))
                ost = osp.tile([128, CHUNK], F32, tag="ost")
                nc.vector.tensor_tensor(ost, ps, bias_b, A.add)
                nc.gpsimd.dma_start(out=out_ap[row:row + 128, col:col + CHUNK],
                                    in_=ost)

            # entry dead after its last burst: free the dict slot and load the
            # next ring entry (its sub-DMAs WAR-wait on this entry's matmuls)
            if _LAST_USE[ent] == pos:
                del wtcs[ent]
                nxt = ent + WC_ENT
                if nxt < len(ENTRY_CH):
                    wtcs[nxt] = load_wtc(ENTRY_CH[nxt])


def _get_nc():
    if "nc" not in _CACHE:
        _CACHE["nc"] = _build_bass()
    return _CACHE["nc"]


def _prep_in_maps(x, weight, bias, q_idx, fp_idx):
    x = np.ascontiguousarray(np.asarray(x, dtype=np.float32)).reshape(S_FULL, D)
    weight = np.asarray(weight, dtype=np.float32)
    bias = np.asarray(bias, dtype=np.float32)
    q_idx = np.asarray(q_idx).astype(np.int64)
    fp_idx = np.asarray(fp_idx).astype(np.int64)

    perm = np.concatenate([q_idx, fp_idx])
    xp = np.ascontiguousarray(x[:, perm])
    wt = np.ascontiguousarray(weight[:, perm].T.astype(ml_dtypes.bfloat16))

    shared = {"wt": wt, "biasf": np.ascontiguousarray(bias[None, :])}
    return [
        {"x": np.ascontiguousarray(xp[c * T:(c + 1) * T]), **shared}
        for c in range(N_CORES)
    ]


def kernel(x, weight, bias, q_idx, fp_idx):
    from concourse import bass_utils
    bass_utils.upload_artifacts = lambda tmpdir: "local://none"

    nc = _get_nc()
    in_maps = _prep_in_maps(x, weight, bias, q_idx, fp_idx)
    res = bass_utils.run_bass_kernel_spmd(
        nc, in_maps, core_ids=list(range(N_CORES)))
    out = np.concatenate([res.results[c]["out"] for c in range(N_CORES)], axis=0)
    return out.reshape(1, S_FULL, O)
